# revision 3
# baseline (speedup 1.0000x reference)
"""Paged-attention decode (GQA 32q/8kv heads, HD=128, paged KV cache) on 8 TRN2 NeuronCores.

Sharding: by sequence (8 seqs per core). Host pre-gathers each sequence's KV-cache
blocks (the sequence "owns its blocks"), storing K d-major (the vLLM-style cache
layout trick) so the device never transposes. Device streams 128-token chunks:
  scoresT[s,g] = KT_chunk_h^T-free matmul (K stationary, q moving)  -> PSUM [128,32]
  probsT = exp(scoresT)                                             -> ACT
  AV: probsT slices stationary, V natural-layout moving             -> PSUM [4,512]x2
  denom: probsT stationary, per-chunk mask column moving            -> PSUM [32,1]
Ragged contexts handled by: per-seq chunk counts known at graph-build time,
position-wise envelope padding across cores (zero chunks are exact no-ops:
exp(0)=1 probs hit V=0, mask=0 kills the denominator).
"""

import os
import sys

for _p in ("/opt/trn_rl_repo", "/opt/pypackages"):
    if _p not in sys.path and os.path.isdir(_p):
        sys.path.append(_p)

import numpy as np

import concourse.mybir as mybir
import concourse.tile as tile
from concourse import bacc
from concourse.bass_utils import run_bass_kernel_spmd

# problem constants (hardcoded per harness contract)
B, H, KV, HD = 64, 32, 8, 128
BS, MAXC = 16, 2048
MB = MAXC // BS
NB = B * MB
SCALE = HD ** -0.5
N_CORES = 8
SPC = B // N_CORES  # sequences per core
CH = 128            # tokens per chunk
G = H // KV         # GQA group size

F32 = mybir.dt.float32

_GRAPH_CACHE: dict = {}
LAST_EXEC_NS = None


def _maybe_install_ntff_hook():
    """Best-effort shim for antenv.axon_hooks so BASS_TRACE=1 profiling works."""
    try:
        import antenv.axon_hooks  # noqa: F401
        return
    except ImportError:
        pass
    try:
        import types
        import antenv
        bp = "/root/.axon_site/trn_agent_boot"
        if bp not in sys.path and os.path.isdir(bp):
            sys.path.append(bp)
        import trn_boot
        hook = trn_boot._ntff_profile_via_ctypes("/opt/axon/libaxon_pjrt.so")
        mod = types.ModuleType("antenv.axon_hooks")
        mod.get_axon_ntff_profile_hook = lambda: hook
        mod.set_axon_ntff_profile_hook = lambda h: None
        antenv.axon_hooks = mod
        sys.modules["antenv.axon_hooks"] = mod
    except Exception:
        pass


def _build_graph(counts: tuple):
    """counts[j] = number of 128-token chunks processed for sequence slot j (same on
    every core -- the position-wise envelope)."""
    T = int(sum(counts))
    nc = bacc.Bacc(None, target_bir_lowering=False)
    kt_d = nc.dram_tensor("kt", [T, HD, KV * CH], F32, kind="ExternalInput")
    vt_d = nc.dram_tensor("vt", [T, CH, KV * HD], F32, kind="ExternalInput")
    qt_d = nc.dram_tensor("qt", [HD, SPC * H], F32, kind="ExternalInput")
    mk_d = nc.dram_tensor("mk", [CH, T], F32, kind="ExternalInput")
    out_d = nc.dram_tensor("out", [SPC, H, HD], F32, kind="ExternalOutput")

    from contextlib import ExitStack

    with tile.TileContext(nc) as tc, ExitStack() as ctx:
        ktp = ctx.enter_context(tc.tile_pool(name="ktp", bufs=4))
        vtp = ctx.enter_context(tc.tile_pool(name="vtp", bufs=4))
        sing = ctx.enter_context(tc.tile_pool(name="sing", bufs=1))
        prp = ctx.enter_context(tc.tile_pool(name="prp", bufs=3))
        epp = ctx.enter_context(tc.tile_pool(name="epp", bufs=2))
        ps_sc = ctx.enter_context(tc.tile_pool(name="ps_sc", bufs=2, space="PSUM"))
        ps_av = ctx.enter_context(tc.tile_pool(name="ps_av", bufs=4, space="PSUM"))
        ps_dn = ctx.enter_context(tc.tile_pool(name="ps_dn", bufs=2, space="PSUM"))
        drp = ctx.enter_context(tc.tile_pool(name="drp", bufs=2, space="DRAM"))

        qt = sing.tile([HD, SPC * H], F32)
        nc.sync.dma_start(out=qt, in_=qt_d[:])
        mk = sing.tile([CH, T], F32)
        nc.sync.dma_start(out=mk, in_=mk_d[:])

        t = 0
        for j in range(SPC):
            nch = int(counts[j])
            av0 = ps_av.tile([G, 4 * HD], F32, tag="av")
            av1 = ps_av.tile([G, 4 * HD], F32, tag="av")
            avps = [av0, av1]
            dnps = ps_dn.tile([H, 1], F32)
            for c in range(nch):
                kt_t = ktp.tile([HD, KV * CH], F32)
                nc.sync.dma_start(out=kt_t, in_=kt_d[t])
                vt_t = vtp.tile([CH, KV * HD], F32)
                nc.sync.dma_start(out=vt_t, in_=vt_d[t])

                scps = ps_sc.tile([CH, H], F32)
                for h in range(KV):
                    # scoresT[s, g] = sum_d KT[d, s] * qT[d, g]
                    # start zeroes the whole 2KB PSUM zero-region (bank), so only
                    # the first matmul into this bank may set it
                    nc.tensor.matmul(
                        scps[:, G * h:G * h + G],
                        kt_t[:, CH * h:CH * (h + 1)],
                        qt[:, H * j + G * h: H * j + G * h + G],
                        start=(h == 0), stop=(h == KV - 1),
                    )
                probs = prp.tile([CH, H], F32)
                nc.scalar.activation(probs, scps, mybir.ActivationFunctionType.Exp)

                st = c == 0
                sp = c == nch - 1
                for h in range(KV):
                    # av[g, d] += sum_s probsT[s, g] * V[s, d]
                    # each [4,512] bank: start only on its first head of chunk 0,
                    # stop only on its last head of the last chunk
                    nc.tensor.matmul(
                        avps[h // 4][:, HD * (h % 4):HD * (h % 4 + 1)],
                        probs[:, G * h:G * h + G],
                        vt_t[:, HD * h:HD * (h + 1)],
                        start=(st and h % 4 == 0), stop=(sp and h % 4 == 3),
                    )
                # den[qh] += sum_s probsT[s, qh] * mask[s]
                nc.tensor.matmul(dnps, probs, mk[:, t:t + 1], start=st, stop=sp)
                t += 1

            # per-sequence epilogue
            av_sb = epp.tile([G, KV * HD], F32, tag="av_sb")
            nc.vector.tensor_copy(av_sb[:, 0:4 * HD], avps[0])
            nc.vector.tensor_copy(av_sb[:, 4 * HD:8 * HD], avps[1])
            den_sb = epp.tile([H, 1], F32, tag="den_sb")
            nc.vector.tensor_copy(den_sb, dnps)
            rden = epp.tile([H, 1], F32, tag="rden")
            nc.vector.reciprocal(rden, den_sb)
            # remap [g, (h d)] -> [(h g), d] through DRAM bounce
            av_bounce = drp.tile([H, HD], F32, tag="avb")
            nc.sync.dma_start(
                out=av_bounce.rearrange("(h g) d -> g h d", g=G),
                in_=av_sb.rearrange("g (h d) -> g h d", h=KV),
            )
            out_sb = epp.tile([H, HD], F32, tag="out_sb")
            nc.sync.dma_start(out=out_sb, in_=av_bounce[:])
            nc.vector.tensor_scalar_mul(out_sb, out_sb, rden)
            nc.sync.dma_start(out=out_d[j], in_=out_sb)

    nc.finalize()
    return nc


def _get_graph(counts):
    key = tuple(int(x) for x in counts)
    if key not in _GRAPH_CACHE:
        _GRAPH_CACHE[key] = _build_graph(key)
    return _GRAPH_CACHE[key]


def kernel(q, k, v, k_cache, v_cache, slot_mapping, block_tables, context_lens):
    global LAST_EXEC_NS
    if os.environ.get("BASS_TRACE"):
        _maybe_install_ntff_hook()

    q = np.asarray(q, dtype=np.float32)
    k = np.asarray(k, dtype=np.float32)
    v = np.asarray(v, dtype=np.float32)
    k_cache = np.asarray(k_cache, dtype=np.float32)
    v_cache = np.asarray(v_cache, dtype=np.float32)
    block_tables = np.asarray(block_tables)
    ctx = np.asarray(context_lens).astype(np.int64)

    cnt = -(-ctx // CH)  # ceil(ctx / CH), >= 1 since ctx >= 1

    # LPT bin-packing of sequences onto cores (8 each), minimizing chunk loads
    order = np.argsort(-cnt, kind="stable")
    cores = [[] for _ in range(N_CORES)]
    loads = [0] * N_CORES
    for s in order:
        cand = [c for c in range(N_CORES) if len(cores[c]) < SPC]
        c = min(cand, key=lambda cc: loads[cc])
        cores[c].append(int(s))
        loads[c] += int(cnt[s])
    for c in range(N_CORES):
        cores[c].sort(key=lambda s: -int(cnt[s]))
    counts = tuple(
        max(int(cnt[cores[c][j]]) for c in range(N_CORES)) for j in range(SPC)
    )
    T = int(sum(counts))
    offs = np.concatenate([[0], np.cumsum(counts)]).astype(int)

    nc = _get_graph(counts)

    kf = k_cache.reshape(NB * BS, KV, HD)
    vf = v_cache.reshape(NB * BS, KV, HD)

    in_maps = []
    for c in range(N_CORES):
        kt = np.zeros((T, HD, KV * CH), np.float32)
        vt = np.zeros((T, CH, KV * HD), np.float32)
        mk = np.zeros((CH, T), np.float32)
        qt = np.zeros((HD, SPC * H), np.float32)
        for j in range(SPC):
            b = cores[c][j]
            n = int(cnt[b])
            L = int(ctx[b])
            S = n * CH
            t0 = offs[j]
            pos = np.arange(S)
            slots = block_tables[b, pos // BS].astype(np.int64) * BS + pos % BS
            Kg = kf[slots]
            Vg = vf[slots]
            Kg[L - 1] = k[b]
            Vg[L - 1] = v[b]
            if L < S:
                Kg[L:] = 0.0
                Vg[L:] = 0.0
            kt[t0:t0 + n] = (
                Kg.reshape(n, CH, KV, HD).transpose(0, 3, 2, 1).reshape(n, HD, KV * CH)
            )
            vt[t0:t0 + n] = Vg.reshape(n, CH, KV * HD)
            m = np.zeros(S, np.float32)
            m[:L] = 1.0
            mk[:, t0:t0 + n] = m.reshape(n, CH).T
            qt[:, j * H:(j + 1) * H] = (q[b] * SCALE).T
        in_maps.append({"kt": kt, "vt": vt, "qt": qt, "mk": mk})

    res = run_bass_kernel_spmd(nc, in_maps, core_ids=list(range(N_CORES)))
    LAST_EXEC_NS = res.exec_time_ns

    out = np.zeros((B, 1, H, HD), np.float32)
    for c in range(N_CORES):
        o = res.results[c]["out"]
        for j in range(SPC):
            out[cores[c][j], 0] = o[j]
    return out


# revision 5
# speedup vs baseline: 1.6449x; 1.6449x over previous
"""Paged-attention decode (GQA 32q/8kv heads, HD=128, paged KV cache) on 8 TRN2 NeuronCores.

Sharding: by sequence (8 seqs per core). Host pre-gathers each sequence's KV-cache
blocks (the sequence "owns its blocks"), storing K d-major (the vLLM-style cache
layout trick) so the device never transposes. Device streams 128-token chunks:
  scoresT[s,g] = KT_chunk_h^T-free matmul (K stationary, q moving)  -> PSUM [128,32]
  probsT = exp(scoresT)                                             -> ACT
  AV: probsT slices stationary, V natural-layout moving             -> PSUM [4,512]x2
  denom: probsT stationary, per-chunk mask column moving            -> PSUM [32,1]
Ragged contexts handled by: per-seq chunk counts known at graph-build time,
position-wise envelope padding across cores (zero chunks are exact no-ops:
exp(0)=1 probs hit V=0, mask=0 kills the denominator).
"""

import os
import sys

for _p in ("/opt/trn_rl_repo", "/opt/pypackages"):
    if _p not in sys.path and os.path.isdir(_p):
        sys.path.append(_p)

import ml_dtypes
import numpy as np

import concourse.mybir as mybir
import concourse.tile as tile
from concourse import bacc
from concourse.bass_utils import run_bass_kernel_spmd

# problem constants (hardcoded per harness contract)
B, H, KV, HD = 64, 32, 8, 128
BS, MAXC = 16, 2048
MB = MAXC // BS
NB = B * MB
SCALE = HD ** -0.5
N_CORES = 8
SPC = B // N_CORES  # sequences per core
CH = 128            # tokens per chunk
G = H // KV         # GQA group size

F32 = mybir.dt.float32
BF16 = mybir.dt.bfloat16

_GRAPH_CACHE: dict = {}
LAST_EXEC_NS = None


def _maybe_install_ntff_hook():
    """Best-effort shim for antenv.axon_hooks so BASS_TRACE=1 profiling works."""
    try:
        import antenv.axon_hooks  # noqa: F401
        return
    except ImportError:
        pass
    try:
        import types
        import antenv
        bp = "/root/.axon_site/trn_agent_boot"
        if bp not in sys.path and os.path.isdir(bp):
            sys.path.append(bp)
        import trn_boot
        hook = trn_boot._ntff_profile_via_ctypes("/opt/axon/libaxon_pjrt.so")
        mod = types.ModuleType("antenv.axon_hooks")
        mod.get_axon_ntff_profile_hook = lambda: hook
        mod.set_axon_ntff_profile_hook = lambda h: None
        antenv.axon_hooks = mod
        sys.modules["antenv.axon_hooks"] = mod
    except Exception:
        pass


def _build_graph(counts: tuple):
    """counts[j] = number of 128-token chunks processed for sequence slot j (same on
    every core -- the position-wise envelope)."""
    T = int(sum(counts))
    nc = bacc.Bacc(None, target_bir_lowering=False)
    kt_d = nc.dram_tensor("kt", [T, HD, KV * CH], F32, kind="ExternalInput")
    vt_d = nc.dram_tensor("vt", [T, CH, KV * HD], F32, kind="ExternalInput")
    qt_d = nc.dram_tensor("qt", [HD, SPC * H], BF16, kind="ExternalInput")
    mk_d = nc.dram_tensor("mk", [CH, T], BF16, kind="ExternalInput")
    out_d = nc.dram_tensor("out", [SPC, H, HD], F32, kind="ExternalOutput")

    from contextlib import ExitStack

    with tile.TileContext(nc) as tc, ExitStack() as ctx:
        ktp = ctx.enter_context(tc.tile_pool(name="ktp", bufs=4))
        vtp = ctx.enter_context(tc.tile_pool(name="vtp", bufs=4))
        sing = ctx.enter_context(tc.tile_pool(name="sing", bufs=1))
        prp = ctx.enter_context(tc.tile_pool(name="prp", bufs=3))
        kt16p = ctx.enter_context(tc.tile_pool(name="kt16p", bufs=3))
        vt16p = ctx.enter_context(tc.tile_pool(name="vt16p", bufs=3))
        epp = ctx.enter_context(tc.tile_pool(name="epp", bufs=2))
        ps_sc = ctx.enter_context(tc.tile_pool(name="ps_sc", bufs=2, space="PSUM"))
        ps_av = ctx.enter_context(tc.tile_pool(name="ps_av", bufs=4, space="PSUM"))
        ps_dn = ctx.enter_context(tc.tile_pool(name="ps_dn", bufs=2, space="PSUM"))
        drp = ctx.enter_context(tc.tile_pool(name="drp", bufs=2, space="DRAM"))

        qt = sing.tile([HD, SPC * H], BF16)
        nc.sync.dma_start(out=qt, in_=qt_d[:])
        mk = sing.tile([CH, T], BF16)
        nc.sync.dma_start(out=mk, in_=mk_d[:])

        t = 0
        for j in range(SPC):
            nch = int(counts[j])
            av0 = ps_av.tile([G, 4 * HD], F32, tag="av")
            av1 = ps_av.tile([G, 4 * HD], F32, tag="av")
            avps = [av0, av1]
            dnps = ps_dn.tile([H, 1], F32)
            for c in range(nch):
                kt_f = ktp.tile([HD, KV * CH], F32)
                nc.sync.dma_start(out=kt_f, in_=kt_d[t])
                vt_f = vtp.tile([CH, KV * HD], F32)
                nc.sync.dma_start(out=vt_f, in_=vt_d[t])
                # downcast to bf16 on-chip: single-pass matmuls + FWL weight loads
                kt_t = kt16p.tile([HD, KV * CH], BF16)
                nc.scalar.activation(kt_t, kt_f, mybir.ActivationFunctionType.Copy)
                vt_t = vt16p.tile([CH, KV * HD], BF16)
                nc.vector.tensor_copy(vt_t, vt_f)

                scps = ps_sc.tile([CH, H], F32)
                for h in range(KV):
                    # scoresT[s, g] = sum_d KT[d, s] * qT[d, g]
                    # start zeroes the whole 2KB PSUM zero-region (bank), so only
                    # the first matmul into this bank may set it
                    nc.tensor.matmul(
                        scps[:, G * h:G * h + G],
                        kt_t[:, CH * h:CH * (h + 1)],
                        qt[:, H * j + G * h: H * j + G * h + G],
                        start=(h == 0), stop=(h == KV - 1),
                    )
                probs = prp.tile([CH, H], BF16)
                nc.scalar.activation(probs, scps, mybir.ActivationFunctionType.Exp)

                st = c == 0
                sp = c == nch - 1
                for h in range(KV):
                    # av[g, d] += sum_s probsT[s, g] * V[s, d]
                    # each [4,512] bank: start only on its first head of chunk 0,
                    # stop only on its last head of the last chunk
                    nc.tensor.matmul(
                        avps[h // 4][:, HD * (h % 4):HD * (h % 4 + 1)],
                        probs[:, G * h:G * h + G],
                        vt_t[:, HD * h:HD * (h + 1)],
                        start=(st and h % 4 == 0), stop=(sp and h % 4 == 3),
                    )
                # den[qh] += sum_s probsT[s, qh] * mask[s]
                nc.tensor.matmul(dnps, probs, mk[:, t:t + 1], start=st, stop=sp)
                t += 1

            # per-sequence epilogue
            av_sb = epp.tile([G, KV * HD], F32, tag="av_sb")
            nc.vector.tensor_copy(av_sb[:, 0:4 * HD], avps[0])
            nc.vector.tensor_copy(av_sb[:, 4 * HD:8 * HD], avps[1])
            den_sb = epp.tile([H, 1], F32, tag="den_sb")
            nc.vector.tensor_copy(den_sb, dnps)
            rden = epp.tile([H, 1], F32, tag="rden")
            nc.vector.reciprocal(rden, den_sb)
            # remap [g, (h d)] -> [(h g), d] through DRAM bounce
            av_bounce = drp.tile([H, HD], F32, tag="avb")
            nc.sync.dma_start(
                out=av_bounce.rearrange("(h g) d -> g h d", g=G),
                in_=av_sb.rearrange("g (h d) -> g h d", h=KV),
            )
            out_sb = epp.tile([H, HD], F32, tag="out_sb")
            nc.sync.dma_start(out=out_sb, in_=av_bounce[:])
            nc.vector.tensor_scalar_mul(out_sb, out_sb, rden)
            nc.sync.dma_start(out=out_d[j], in_=out_sb)

    nc.finalize()
    return nc


def _get_graph(counts):
    key = tuple(int(x) for x in counts)
    if key not in _GRAPH_CACHE:
        _GRAPH_CACHE[key] = _build_graph(key)
    return _GRAPH_CACHE[key]


def kernel(q, k, v, k_cache, v_cache, slot_mapping, block_tables, context_lens):
    global LAST_EXEC_NS
    if os.environ.get("BASS_TRACE"):
        _maybe_install_ntff_hook()

    q = np.asarray(q, dtype=np.float32)
    k = np.asarray(k, dtype=np.float32)
    v = np.asarray(v, dtype=np.float32)
    k_cache = np.asarray(k_cache, dtype=np.float32)
    v_cache = np.asarray(v_cache, dtype=np.float32)
    block_tables = np.asarray(block_tables)
    ctx = np.asarray(context_lens).astype(np.int64)

    cnt = -(-ctx // CH)  # ceil(ctx / CH), >= 1 since ctx >= 1

    # LPT bin-packing of sequences onto cores (8 each), minimizing chunk loads
    order = np.argsort(-cnt, kind="stable")
    cores = [[] for _ in range(N_CORES)]
    loads = [0] * N_CORES
    for s in order:
        cand = [c for c in range(N_CORES) if len(cores[c]) < SPC]
        c = min(cand, key=lambda cc: loads[cc])
        cores[c].append(int(s))
        loads[c] += int(cnt[s])
    for c in range(N_CORES):
        cores[c].sort(key=lambda s: -int(cnt[s]))
    counts = tuple(
        max(int(cnt[cores[c][j]]) for c in range(N_CORES)) for j in range(SPC)
    )
    T = int(sum(counts))
    offs = np.concatenate([[0], np.cumsum(counts)]).astype(int)

    nc = _get_graph(counts)

    kf = k_cache.reshape(NB * BS, KV, HD)
    vf = v_cache.reshape(NB * BS, KV, HD)

    in_maps = []
    for c in range(N_CORES):
        kt = np.zeros((T, HD, KV * CH), np.float32)
        vt = np.zeros((T, CH, KV * HD), np.float32)
        mk = np.zeros((CH, T), ml_dtypes.bfloat16)
        qt = np.zeros((HD, SPC * H), ml_dtypes.bfloat16)
        for j in range(SPC):
            b = cores[c][j]
            n = int(cnt[b])
            L = int(ctx[b])
            S = n * CH
            t0 = offs[j]
            pos = np.arange(S)
            slots = block_tables[b, pos // BS].astype(np.int64) * BS + pos % BS
            Kg = kf[slots]
            Vg = vf[slots]
            Kg[L - 1] = k[b]
            Vg[L - 1] = v[b]
            if L < S:
                Kg[L:] = 0.0
                Vg[L:] = 0.0
            kt[t0:t0 + n] = (
                Kg.reshape(n, CH, KV, HD).transpose(0, 3, 2, 1).reshape(n, HD, KV * CH)
            )
            vt[t0:t0 + n] = Vg.reshape(n, CH, KV * HD)
            m = np.zeros(S, ml_dtypes.bfloat16)
            m[:L] = 1.0
            mk[:, t0:t0 + n] = m.reshape(n, CH).T
            qt[:, j * H:(j + 1) * H] = (q[b] * SCALE).T
        in_maps.append({"kt": kt, "vt": vt, "qt": qt, "mk": mk})

    res = run_bass_kernel_spmd(nc, in_maps, core_ids=list(range(N_CORES)))
    LAST_EXEC_NS = res.exec_time_ns

    out = np.zeros((B, 1, H, HD), np.float32)
    for c in range(N_CORES):
        o = res.results[c]["out"]
        for j in range(SPC):
            out[cores[c][j], 0] = o[j]
    return out


# revision 7
# speedup vs baseline: 2.0330x; 1.2359x over previous
"""Paged-attention decode (GQA 32q/8kv heads, HD=128, paged KV cache) on 8 TRN2 NeuronCores.

Sharding: by sequence (8 seqs per core). Host pre-gathers each sequence's KV-cache
blocks (the sequence "owns its blocks"), storing K d-major (the vLLM-style cache
layout trick) so the device never transposes. Device streams 128-token chunks:
  scoresT[s,g] = KT_chunk_h^T-free matmul (K stationary, q moving)  -> PSUM [128,32]
  probsT = exp(scoresT)                                             -> ACT
  AV: probsT slices stationary, V natural-layout moving             -> PSUM [4,512]x2
  denom: probsT stationary, per-chunk mask column moving            -> PSUM [32,1]
Ragged contexts handled by: per-seq chunk counts known at graph-build time,
position-wise envelope padding across cores (zero chunks are exact no-ops:
exp(0)=1 probs hit V=0, mask=0 kills the denominator).
"""

import os
import sys

for _p in ("/opt/trn_rl_repo", "/opt/pypackages"):
    if _p not in sys.path and os.path.isdir(_p):
        sys.path.append(_p)

import ml_dtypes
import numpy as np

import concourse.mybir as mybir
import concourse.tile as tile
from concourse import bacc
from concourse.bass_utils import run_bass_kernel_spmd

# problem constants (hardcoded per harness contract)
B, H, KV, HD = 64, 32, 8, 128
BS, MAXC = 16, 2048
MB = MAXC // BS
NB = B * MB
SCALE = HD ** -0.5
N_CORES = 8
SPC = B // N_CORES  # sequences per core
CH = 128            # tokens per chunk
G = H // KV         # GQA group size

F32 = mybir.dt.float32
BF16 = mybir.dt.bfloat16

_GRAPH_CACHE: dict = {}
LAST_EXEC_NS = None


def _maybe_install_ntff_hook():
    """Best-effort shim for antenv.axon_hooks so BASS_TRACE=1 profiling works."""
    try:
        import antenv.axon_hooks  # noqa: F401
        return
    except ImportError:
        pass
    try:
        import types
        import antenv
        bp = "/root/.axon_site/trn_agent_boot"
        if bp not in sys.path and os.path.isdir(bp):
            sys.path.append(bp)
        import trn_boot
        hook = trn_boot._ntff_profile_via_ctypes("/opt/axon/libaxon_pjrt.so")
        mod = types.ModuleType("antenv.axon_hooks")
        mod.get_axon_ntff_profile_hook = lambda: hook
        mod.set_axon_ntff_profile_hook = lambda h: None
        antenv.axon_hooks = mod
        sys.modules["antenv.axon_hooks"] = mod
    except Exception:
        pass


GRP = 4  # chunks per DMA group (4 MiB per dma_start)
CW = 2 * KV * CH  # 2048 f32 columns per chunk in the merged kv stream


def _build_graph(counts: tuple):
    """counts[j] = number of 128-token chunks processed for sequence slot j (same on
    every core -- the position-wise envelope)."""
    T = int(sum(counts))
    nc = bacc.Bacc(None, target_bir_lowering=False)
    # merged K/V stream: row p = per-chunk [KT row | V row], any chunk group is a
    # contiguous per-partition run -> large efficient DMAs
    kv_d = nc.dram_tensor("kv", [128, T * CW], F32, kind="ExternalInput")
    qt_d = nc.dram_tensor("qt", [HD, SPC * H], BF16, kind="ExternalInput")
    mk_d = nc.dram_tensor("mk", [CH, T], BF16, kind="ExternalInput")
    out_d = nc.dram_tensor("out", [SPC, H, HD], F32, kind="ExternalOutput")

    from contextlib import ExitStack

    with tile.TileContext(nc) as tc, ExitStack() as ctx:
        kvp = ctx.enter_context(tc.tile_pool(name="kvp", bufs=3))
        sing = ctx.enter_context(tc.tile_pool(name="sing", bufs=1))
        prp = ctx.enter_context(tc.tile_pool(name="prp", bufs=3))
        kt16p = ctx.enter_context(tc.tile_pool(name="kt16p", bufs=4))
        vt16p = ctx.enter_context(tc.tile_pool(name="vt16p", bufs=4))
        epp = ctx.enter_context(tc.tile_pool(name="epp", bufs=2))
        ps_sc = ctx.enter_context(tc.tile_pool(name="ps_sc", bufs=2, space="PSUM"))
        ps_av = ctx.enter_context(tc.tile_pool(name="ps_av", bufs=4, space="PSUM"))
        ps_dn = ctx.enter_context(tc.tile_pool(name="ps_dn", bufs=2, space="PSUM"))
        drp = ctx.enter_context(tc.tile_pool(name="drp", bufs=2, space="DRAM"))

        qt = sing.tile([HD, SPC * H], BF16)
        nc.sync.dma_start(out=qt, in_=qt_d[:])
        mk = sing.tile([CH, T], BF16)
        nc.sync.dma_start(out=mk, in_=mk_d[:])

        t = 0
        for j in range(SPC):
            nch = int(counts[j])
            av0 = ps_av.tile([G, 4 * HD], F32, tag="av")
            av1 = ps_av.tile([G, 4 * HD], F32, tag="av")
            avps = [av0, av1]
            dnps = ps_dn.tile([H, 1], F32)
            kv_t = None
            for c in range(nch):
                gi = c % GRP
                if gi == 0:
                    g = min(GRP, nch - c)
                    kv_t = kvp.tile([128, GRP * CW], F32, tag="kv")
                    nc.sync.dma_start(
                        out=kv_t[:, :g * CW],
                        in_=kv_d[:, t * CW:(t + g) * CW],
                    )
                kt_f = kv_t[:, gi * CW: gi * CW + KV * CH]
                vt_f = kv_t[:, gi * CW + KV * CH: (gi + 1) * CW]
                # downcast to bf16 on-chip: single-pass matmuls + FWL weight loads
                kt_t = kt16p.tile([HD, KV * CH], BF16)
                nc.scalar.activation(kt_t, kt_f, mybir.ActivationFunctionType.Copy)
                vt_t = vt16p.tile([CH, KV * HD], BF16)
                nc.vector.tensor_copy(vt_t, vt_f)

                scps = ps_sc.tile([CH, H], F32)
                for h in range(KV):
                    # scoresT[s, g] = sum_d KT[d, s] * qT[d, g]
                    # start zeroes the whole 2KB PSUM zero-region (bank), so only
                    # the first matmul into this bank may set it
                    nc.tensor.matmul(
                        scps[:, G * h:G * h + G],
                        kt_t[:, CH * h:CH * (h + 1)],
                        qt[:, H * j + G * h: H * j + G * h + G],
                        start=(h == 0), stop=(h == KV - 1),
                    )
                probs = prp.tile([CH, H], BF16)
                nc.scalar.activation(probs, scps, mybir.ActivationFunctionType.Exp)

                st = c == 0
                sp = c == nch - 1
                for h in range(KV):
                    # av[g, d] += sum_s probsT[s, g] * V[s, d]
                    # each [4,512] bank: start only on its first head of chunk 0,
                    # stop only on its last head of the last chunk
                    nc.tensor.matmul(
                        avps[h // 4][:, HD * (h % 4):HD * (h % 4 + 1)],
                        probs[:, G * h:G * h + G],
                        vt_t[:, HD * h:HD * (h + 1)],
                        start=(st and h % 4 == 0), stop=(sp and h % 4 == 3),
                    )
                # den[qh] += sum_s probsT[s, qh] * mask[s]
                nc.tensor.matmul(dnps, probs, mk[:, t:t + 1], start=st, stop=sp)
                t += 1

            # per-sequence epilogue
            av_sb = epp.tile([G, KV * HD], F32, tag="av_sb")
            nc.vector.tensor_copy(av_sb[:, 0:4 * HD], avps[0])
            nc.vector.tensor_copy(av_sb[:, 4 * HD:8 * HD], avps[1])
            den_sb = epp.tile([H, 1], F32, tag="den_sb")
            nc.vector.tensor_copy(den_sb, dnps)
            rden = epp.tile([H, 1], F32, tag="rden")
            nc.vector.reciprocal(rden, den_sb)
            # remap [g, (h d)] -> [(h g), d] through DRAM bounce
            av_bounce = drp.tile([H, HD], F32, tag="avb")
            nc.sync.dma_start(
                out=av_bounce.rearrange("(h g) d -> g h d", g=G),
                in_=av_sb.rearrange("g (h d) -> g h d", h=KV),
            )
            out_sb = epp.tile([H, HD], F32, tag="out_sb")
            nc.sync.dma_start(out=out_sb, in_=av_bounce[:])
            nc.vector.tensor_scalar_mul(out_sb, out_sb, rden)
            nc.sync.dma_start(out=out_d[j], in_=out_sb)

    nc.finalize()
    return nc


def _get_graph(counts):
    key = tuple(int(x) for x in counts)
    if key not in _GRAPH_CACHE:
        _GRAPH_CACHE[key] = _build_graph(key)
    return _GRAPH_CACHE[key]


def kernel(q, k, v, k_cache, v_cache, slot_mapping, block_tables, context_lens):
    global LAST_EXEC_NS
    if os.environ.get("BASS_TRACE"):
        _maybe_install_ntff_hook()

    q = np.asarray(q, dtype=np.float32)
    k = np.asarray(k, dtype=np.float32)
    v = np.asarray(v, dtype=np.float32)
    k_cache = np.asarray(k_cache, dtype=np.float32)
    v_cache = np.asarray(v_cache, dtype=np.float32)
    block_tables = np.asarray(block_tables)
    ctx = np.asarray(context_lens).astype(np.int64)

    cnt = -(-ctx // CH)  # ceil(ctx / CH), >= 1 since ctx >= 1

    # LPT bin-packing of sequences onto cores (8 each), minimizing chunk loads
    order = np.argsort(-cnt, kind="stable")
    cores = [[] for _ in range(N_CORES)]
    loads = [0] * N_CORES
    for s in order:
        cand = [c for c in range(N_CORES) if len(cores[c]) < SPC]
        c = min(cand, key=lambda cc: loads[cc])
        cores[c].append(int(s))
        loads[c] += int(cnt[s])
    for c in range(N_CORES):
        cores[c].sort(key=lambda s: -int(cnt[s]))
    counts = tuple(
        max(int(cnt[cores[c][j]]) for c in range(N_CORES)) for j in range(SPC)
    )
    T = int(sum(counts))
    offs = np.concatenate([[0], np.cumsum(counts)]).astype(int)

    nc = _get_graph(counts)

    kf = k_cache.reshape(NB * BS, KV, HD)
    vf = v_cache.reshape(NB * BS, KV, HD)

    in_maps = []
    for c in range(N_CORES):
        kv = np.zeros((128, T, 2, KV * CH), np.float32)  # view of [128, T*CW]
        mk = np.zeros((CH, T), ml_dtypes.bfloat16)
        qt = np.zeros((HD, SPC * H), ml_dtypes.bfloat16)
        for j in range(SPC):
            b = cores[c][j]
            n = int(cnt[b])
            L = int(ctx[b])
            S = n * CH
            t0 = offs[j]
            pos = np.arange(S)
            slots = block_tables[b, pos // BS].astype(np.int64) * BS + pos % BS
            Kg = kf[slots]
            Vg = vf[slots]
            Kg[L - 1] = k[b]
            Vg[L - 1] = v[b]
            if L < S:
                Kg[L:] = 0.0
                Vg[L:] = 0.0
            # KT half: [d, h, s]; V half: [s, h, d]
            kv[:, t0:t0 + n, 0, :] = (
                Kg.reshape(n, CH, KV, HD)
                .transpose(3, 0, 2, 1)
                .reshape(HD, n, KV * CH)
            )
            kv[:, t0:t0 + n, 1, :] = (
                Vg.reshape(n, CH, KV * HD).transpose(1, 0, 2)
            )
            m = np.zeros(S, ml_dtypes.bfloat16)
            m[:L] = 1.0
            mk[:, t0:t0 + n] = m.reshape(n, CH).T
            qt[:, j * H:(j + 1) * H] = (q[b] * SCALE).T
        in_maps.append({"kv": kv.reshape(128, T * CW), "qt": qt, "mk": mk})

    res = run_bass_kernel_spmd(nc, in_maps, core_ids=list(range(N_CORES)))
    LAST_EXEC_NS = res.exec_time_ns

    out = np.zeros((B, 1, H, HD), np.float32)
    for c in range(N_CORES):
        o = res.results[c]["out"]
        for j in range(SPC):
            out[cores[c][j], 0] = o[j]
    return out
